# revision 21
# baseline (speedup 1.0000x reference)
"""HRR self-attention Trainium2 kernel.

Math: reference computes, per head (D=128):
    qkv = x @ W_qkv.T ; q,k,v heads
    kv  = irfft(rfft(k) * rfft(v))          # circular conv bind
    kv  = cumsum(kv, axis=seq)
    out = irfft(rfft(kv) * conj(rfft(q)))   # circular corr unbind
    y   = out @ W_o.T

The rfft/irfft along the head dim are linear maps, so they are folded into
W_qkv / W_o on the host: the device computes frequency-domain q,k,v directly
with one GEMM, does the complex bind / cumsum / unbind elementwise (the
cumsum commutes with the irfft), and applies the irfft+output projection as
a second GEMM. Zero extra FLOPs vs the plain projections.

Sharding: 8 cores = 4 batches x 2 head-groups (4 heads each). Each core
emits a partial output projection for its batch; host sums the two
head-group partials per batch.

Frequency packing per head-pair chunk (D=128 -> rfft bins 0..64): lanes
0..62 head A bins 1..63, lanes 63..125 head B bins 1..63, lanes 126/127
carry the real-only DC (re chunk) and Nyquist (im chunk) of heads A/B.
The special lanes need plain elementwise products, not complex ones; the
bind/unbind op ORDER makes that free: the two full-tile multiplies that
produce the correct special-lane values write the destination tiles first,
then the complex combines overwrite lanes [0:126] only. Same-engine program
order resolves the read-before-overwrite hazard. No fixup ops at all.

Inputs are host-packed into flat SBUF-image DRAM layouts so every DMA is a
contiguous per-partition run (max descriptor size, few instructions: each
DMACopy costs ~1.2us of SP issue + 625ns serialized HWDGE generation).
"""

import numpy as np
import ml_dtypes

B, S, M, H = 4, 2048, 1024, 8
D = M // H          # 128
SC = 512            # sequence chunk
NSC = S // SC       # 4
NMI = M // 128      # 8 contraction chunks
NCC = 12            # qkv freq channel chunks per core
NCO = 4             # U channel chunks per core

BF16 = ml_dtypes.bfloat16

# wf SBUF/DRAM image: col = h*6144 + mi*768 + cch*128 + c
WF_GROUPS = {0: [(0,), (1,), (2, 3), (4, 5), (6, 7)],
             1: [(0, 1, 2, 3), (4, 5, 6, 7)]}
# x image: sc0 region cols mi*512+c; rest region cols 4096 + g*6144 +
# idx*1536 + (sc-1)*512 + c
X0_GROUPS = [(0,), (1,), (2, 3), (4, 5), (6, 7)]


# ---------------------------------------------------------------------------
# Host-side weight fusion
# ---------------------------------------------------------------------------

def _head_rows(Wh, F):
    """Wh (D, M) spatial head weights -> (dc, nyq, re, im) freq rows."""
    FW = F @ Wh  # (65, M) complex
    return FW.real[0:1], FW.real[64:65], FW.real[1:64], FW.imag[1:64]


def build_tables(W_qkv, W_o):
    """Per-core (wf_img [128,12288] bf16, wo_img [128,4096] bf16)."""
    W_qkv = np.asarray(W_qkv, dtype=np.float64)
    W_o = np.asarray(W_o, dtype=np.float64)
    F = np.fft.rfft(np.eye(D), axis=-1).T  # (65, 128)
    Wq = W_qkv[0 * M:1 * M].reshape(H, D, M)
    Wk = W_qkv[1 * M:2 * M].reshape(H, D, M)
    Wv = W_qkv[2 * M:3 * M].reshape(H, D, M)

    # irfft basis columns, packed order [re 1..63 | dc]/[im 1..63 | nyq]
    n = np.arange(D)
    j = np.arange(1, 64)
    Bre = 2.0 * np.cos(2 * np.pi * np.outer(n, j) / D) / D   # (128, 63)
    Bim = -2.0 * np.sin(2 * np.pi * np.outer(n, j) / D) / D  # (128, 63)
    bdc = np.full((D, 1), 1.0 / D)
    bnyq = (np.cos(np.pi * n) / D)[:, None]

    tables = []
    for core in range(8):
        g = core % 2
        heads = [4 * g + i for i in range(4)]
        chunks = []
        out_rows = []
        for pair in range(2):
            hA, hB = heads[2 * pair], heads[2 * pair + 1]
            for Wx in (Wk, Wv, Wq):
                dcA, nyA, reA, imA = _head_rows(Wx[hA], F)
                dcB, nyB, reB, imB = _head_rows(Wx[hB], F)
                chunks.append(np.concatenate([reA, reB, dcA, dcB], axis=0))
                chunks.append(np.concatenate([imA, imB, nyA, nyB], axis=0))
            WoA = W_o[:, D * hA:D * (hA + 1)]  # (1024, 128)
            WoB = W_o[:, D * hB:D * (hB + 1)]
            out_rows.append(np.concatenate(
                [(WoA @ Bre).T, (WoB @ Bre).T, (WoA @ bdc).T, (WoB @ bdc).T],
                axis=0))
            out_rows.append(np.concatenate(
                [(WoA @ Bim).T, (WoB @ Bim).T, (WoA @ bnyq).T, (WoB @ bnyq).T],
                axis=0))
        WfT = np.concatenate(chunks, axis=0).T  # (1024, 1536)
        WoG = np.concatenate(out_rows, axis=0)  # (512, 1024)
        wf_img = np.ascontiguousarray(
            WfT.reshape(8, 128, 2, 768).transpose(1, 2, 0, 3).reshape(128, 12288))
        wo_img = np.ascontiguousarray(
            WoG.reshape(4, 128, 1024).transpose(1, 0, 2).reshape(128, 4096))
        tables.append((wf_img.astype(np.float32).astype(BF16),
                       wo_img.astype(np.float32).astype(BF16)))
    return tables


def pack_x(xb):
    """xb (S, M) f32 -> x image [128, 16384] bf16."""
    xT = np.ascontiguousarray(xb.T).astype(BF16)  # (1024, 2048)
    x0 = xT.reshape(8, 128, 4, 512)[:, :, 0]      # (8, 128, 512)
    x0 = x0.transpose(1, 0, 2).reshape(128, 4096)
    xr = xT[:, 512:].reshape(2, 4, 128, 1536)
    xr = xr.transpose(2, 0, 1, 3).reshape(128, 12288)
    return np.ascontiguousarray(np.concatenate([x0, xr], axis=1))


# ---------------------------------------------------------------------------
# Device kernel
# ---------------------------------------------------------------------------

def build_kernel(tc, xi, wf, wo, out, reps=1, loop_iters=None, salt=""):
    import concourse.mybir as mybir
    from contextlib import ExitStack

    nc = tc.nc
    bf16 = mybir.dt.bfloat16
    f32 = mybir.dt.float32
    MULT = mybir.AluOpType.mult
    ADD = mybir.AluOpType.add

    with ExitStack() as ctx:
        consts = ctx.enter_context(tc.tile_pool(name="consts", bufs=1))
        xpool = ctx.enter_context(tc.tile_pool(name="xpool", bufs=1))
        wpool = ctx.enter_context(tc.tile_pool(name="wpool", bufs=1))
        qkvp = ctx.enter_context(tc.tile_pool(name="qkvp", bufs=2))
        kvp = ctx.enter_context(tc.tile_pool(name="kvp", bufs=2))
        scanp = ctx.enter_context(tc.tile_pool(name="scanp", bufs=2))
        up = ctx.enter_context(tc.tile_pool(name="up", bufs=2))
        tmpp = ctx.enter_context(tc.tile_pool(name="tmpp", bufs=2))
        outp = ctx.enter_context(tc.tile_pool(name="outp", bufs=2))
        psq = ctx.enter_context(tc.tile_pool(name="psq", bufs=4, space="PSUM"))
        psop = ctx.enter_context(tc.tile_pool(name="psop", bufs=4, space="PSUM"))

        ones = consts.tile([128, SC], bf16, name=f"ones{salt}")
        nc.vector.memset(ones[:], 1.0)

        # --- input DMAs: few, contiguous, first-needed first ---
        wf_t = {0: [], 1: []}
        x0_t = []
        xr_t = []

        def wf_dma(h, gi):
            grp = WF_GROUPS[h][gi]
            w = len(grp) * 768
            t = wpool.tile([128, w], bf16, tag=f"wf{h}_{gi}", name=f"wf{h}_{gi}")
            c0 = h * 6144 + grp[0] * 768
            nc.sync.dma_start(out=t[:], in_=wf[:, c0:c0 + w])
            wf_t[h].append(t)

        def x0_dma(gi):
            grp = X0_GROUPS[gi]
            w = len(grp) * 512
            t = xpool.tile([128, w], bf16, tag=f"x0_{gi}", name=f"x0_{gi}")
            c0 = grp[0] * 512
            nc.sync.dma_start(out=t[:], in_=xi[:, c0:c0 + w])
            x0_t.append(t)

        for i in range(max(len(WF_GROUPS[0]), len(X0_GROUPS))):
            if i < len(WF_GROUPS[0]):
                wf_dma(0, i)
            if i < len(X0_GROUPS):
                x0_dma(i)
        for i in range(len(WF_GROUPS[1])):
            wf_dma(1, i)
        for gi in range(2):
            t = xpool.tile([128, 6144], bf16, tag=f"xr{gi}", name=f"xr{gi}")
            c0 = 4096 + gi * 6144
            nc.sync.dma_start(out=t[:], in_=xi[:, c0:c0 + 6144])
            xr_t.append(t)
        wo_t = wpool.tile([128, 4096], bf16, tag="wo", name="wo")
        nc.sync.dma_start(out=wo_t[:], in_=wo[:, :])

        # loop builds: fixed carry tiles let the last chunk's output GEMM
        # move to the TOP of the next For_i iteration (reads the previous
        # iteration's U), so the body never stalls on its own tail chain.
        ucarry = None
        if loop_iters is not None:
            ucarry = [consts.tile([128, SC], bf16, name=f"uc{i}{salt}")
                      for i in range(4)]
            for t in ucarry:
                nc.vector.memset(t[:], 1.0)

        def wf_ap(mi, h, cch):
            for gi, grp in enumerate(WF_GROUPS[h]):
                if mi in grp:
                    c = (mi - grp[0]) * 768 + cch * 128
                    return wf_t[h][gi][:, c:c + 128]
            raise AssertionError

        def x_ap(mi, sc):
            if sc == 0:
                for gi, grp in enumerate(X0_GROUPS):
                    if mi in grp:
                        c = (mi - grp[0]) * 512
                        return x0_t[gi][:, c:c + 512]
                raise AssertionError
            gi, idx = divmod(mi, 4)
            c = idx * 1536 + (sc - 1) * 512
            return xr_t[gi][:, c:c + 512]

        if loop_iters is not None:
            loop_cm = tc.For_i(
                0, loop_iters, 1,
                hint_engines=(mybir.EngineType.PE, mybir.EngineType.DVE,
                              mybir.EngineType.Activation, mybir.EngineType.Pool,
                              mybir.EngineType.SP))
            loop_cm.__enter__()

        CPLX = slice(0, 126)  # complex lanes; 126/127 are real DC/Nyq lanes

        def drain_qkv(rep, sc, cc, ps, chunks, tail):
            sb = qkvp.tile([128, SC], bf16, tag=f"qkv{cc}", name=f"qkv{rep}_{sc}_{cc}")
            # ACT drains keep the DVE free for the bind/scan/unbind chain;
            # on the tail chunk alternate ACT/DVE so the final chain starts
            # sooner (gpsimd has no PSUM port)
            if tail and cc % 2 == 1:
                nc.vector.tensor_copy(sb[:], ps[:])
            else:
                nc.scalar.copy(sb[:], ps[:])
            chunks.append(sb)

        def emit_qkv_vector(rep, sc, prev_scan, tail=False, uout=None):
            chunks = []
            if sc == 0 and loop_iters is None:
                # Startup: x/wf tiles are still streaming in. mi-outer over
                # six h=0 accumulators (4 psq + 2 borrowed psop banks) lets
                # the PE consume each (wf, x) mi-group as it lands instead
                # of stalling inside chunk 0's accumulation.
                ps6 = [psq.tile([128, SC], f32, tag="psq", name=f"psq{rep}_0_{c}")
                       for c in range(4)]
                ps6 += [psop.tile([128, SC], f32, tag="pso", name=f"psq{rep}_0_{c}")
                        for c in range(4, 6)]
                for mi in range(NMI):
                    for cc in range(6):
                        nc.tensor.matmul(
                            ps6[cc][:], wf_ap(mi, 0, cc), x_ap(mi, 0),
                            start=(mi == 0), stop=(mi == NMI - 1))
                for cc in range(6):
                    drain_qkv(rep, sc, cc, ps6[cc], chunks, tail)
                rest = range(6, NCC)
            else:
                rest = range(NCC)
            for cc in rest:
                ps = psq.tile([128, SC], f32, tag="psq", name=f"psq{rep}_{sc}_{cc}")
                h, cch = divmod(cc, 6)
                for mi in range(NMI):
                    nc.tensor.matmul(
                        ps[:], wf_ap(mi, h, cch), x_ap(mi, sc),
                        start=(mi == 0), stop=(mi == NMI - 1))
                drain_qkv(rep, sc, cc, ps, chunks, tail)

            U = []
            for pair in range(2):
                Kre, Kim, Vre, Vim, Qre, Qim = chunks[6 * pair:6 * pair + 6]

                KVre = kvp.tile([128, SC], bf16, tag=f"kvre{pair}", name=f"kvre{sc}_{pair}")
                KVim = kvp.tile([128, SC], bf16, tag=f"kvim{pair}", name=f"kvim{sc}_{pair}")
                t3 = tmpp.tile([128, SC], bf16, tag="t3", name=f"t3_{sc}_{pair}")
                t4 = tmpp.tile([128, SC], bf16, tag="t4", name=f"t4_{sc}_{pair}")
                # lanes 126/127 of KVre/KVim are final after the first two
                # muls (real DC/Nyq products); the complex combines then
                # overwrite lanes [0:126] only. Program order on the DVE
                # queue makes the read-then-overwrite of KVim safe.
                nc.vector.tensor_mul(KVre[:], Kre[:], Vre[:])
                nc.vector.tensor_mul(KVim[:], Kim[:], Vim[:])
                nc.vector.tensor_sub(KVre[CPLX], KVre[CPLX], KVim[CPLX])
                nc.vector.tensor_mul(t3[:], Kre[:], Vim[:])
                nc.vector.tensor_mul(t4[:], Kim[:], Vre[:])
                nc.vector.tensor_add(KVim[CPLX], t3[CPLX], t4[CPLX])

                KVre_c = scanp.tile([128, SC], f32, tag=f"scre{pair}", name=f"scre{sc}_{pair}")
                KVim_c = scanp.tile([128, SC], f32, tag=f"scim{pair}", name=f"scim{sc}_{pair}")
                init_re = 0.0 if sc == 0 else prev_scan[(pair, 0)][:, SC - 1:SC]
                init_im = 0.0 if sc == 0 else prev_scan[(pair, 1)][:, SC - 1:SC]
                nc.vector.tensor_tensor_scan(
                    KVre_c[:], ones[:], KVre[:], init_re, MULT, ADD)
                nc.vector.tensor_tensor_scan(
                    KVim_c[:], ones[:], KVim[:], init_im, MULT, ADD)
                prev_scan[(pair, 0)] = KVre_c
                prev_scan[(pair, 1)] = KVim_c

                if uout is not None:
                    Ure, Uim = uout[2 * pair], uout[2 * pair + 1]
                else:
                    Ure = up.tile([128, SC], bf16, tag=f"ure{pair}", name=f"ure{sc}_{pair}")
                    Uim = up.tile([128, SC], bf16, tag=f"uim{pair}", name=f"uim{sc}_{pair}")
                u3 = tmpp.tile([128, SC], f32, tag="u3", name=f"u3_{sc}_{pair}")
                u4 = tmpp.tile([128, SC], f32, tag="u4", name=f"u4_{sc}_{pair}")
                # same trick: Uim first holds Qim*KVim (lanes 126/127 final),
                # Ure holds Qre*KVre; complex lanes are then overwritten.
                nc.vector.tensor_mul(Uim[:], KVim_c[:], Qim[:])
                nc.vector.tensor_mul(Ure[:], KVre_c[:], Qre[:])
                nc.vector.tensor_add(Ure[CPLX], Ure[CPLX], Uim[CPLX])
                nc.vector.tensor_mul(u3[:], KVim_c[:], Qre[:])
                nc.vector.tensor_mul(u4[:], KVre_c[:], Qim[:])
                nc.vector.tensor_sub(Uim[CPLX], u3[CPLX], u4[CPLX])
                U += [Ure, Uim]
            return U

        def emit_out(rep, sc, U, final=False):
            wide = outp.tile([128, 4096], f32, tag="wide", name=f"wide{rep}_{sc}")
            for half in range(2):
                po = [psop.tile([128, SC], f32, tag="pso",
                                name=f"pso{rep}_{sc}_{half}_{q}") for q in range(4)]
                # ci-outer: the ci=0 matmuls only need U[0], so the PE can
                # start the output GEMM as soon as the first U chunk lands.
                for ci in range(NCO):
                    for q in range(4):
                        mo = half * 4 + q
                        nc.tensor.matmul(
                            po[q][:], wo_t[:, ci * 1024 + mo * 128:ci * 1024 + (mo + 1) * 128],
                            U[ci][:], start=(ci == 0), stop=(ci == NCO - 1))
                if not final:
                    for q in range(4):
                        mo = half * 4 + q
                        nc.scalar.copy(wide[:, mo * 512:(mo + 1) * 512], po[q][:])
                    nc.sync.dma_start(
                        out=out[:, sc * 4096 + half * 2048:sc * 4096 + (half + 1) * 2048],
                        in_=wide[:, half * 2048:(half + 1) * 2048])
                else:
                    # tail drain: alternate copy engines + per-pair DMAs so
                    # the final writeback pipelines instead of serializing.
                    for q2 in range(2):
                        for q in (q2 * 2, q2 * 2 + 1):
                            mo = half * 4 + q
                            if q % 2 == 0:
                                nc.scalar.copy(wide[:, mo * 512:(mo + 1) * 512], po[q][:])
                            else:
                                nc.vector.tensor_copy(wide[:, mo * 512:(mo + 1) * 512], po[q][:])
                        mo0 = half * 4 + q2 * 2
                        nc.sync.dma_start(
                            out=out[:, sc * 4096 + mo0 * 512:sc * 4096 + (mo0 + 2) * 512],
                            in_=wide[:, mo0 * 512:(mo0 + 2) * 512])

        # Software pipelining: emit s-chunk sc+1's qkv matmuls BEFORE s-chunk
        # sc's output matmuls. The PE executes its queue in order, so this
        # keeps it streaming qkv work while the DVE bind/scan/unbind chain for
        # the previous chunk produces U. The pending-out carries ACROSS reps
        # so the PE starts the next rep's qkv instead of stalling on the last
        # chunk's chain at each iteration boundary.
        pend = None
        if ucarry is not None:
            # consumes the PREVIOUS For_i iteration's carry (memset-seeded
            # on the first pass); keeps the PE busy through the loop wrap.
            emit_out(reps - 1, NSC - 1, ucarry, final=True)
        for rep in range(reps):
            prev_scan = {}
            for sc in range(NSC):
                last = (rep == reps - 1 and sc == NSC - 1)
                U = emit_qkv_vector(rep, sc, prev_scan, tail=last,
                                    uout=(ucarry if (last and ucarry is not None)
                                          else None))
                if pend is not None:
                    emit_out(*pend)
                pend = (rep, sc, U)
        if ucarry is None:
            emit_out(*pend, final=True)
        if loop_iters is not None:
            loop_cm.__exit__(None, None, None)


def build_bass(reps=1, loop_iters=None, salt=""):
    import concourse.bacc as bacc
    import concourse.tile as tile
    import concourse.mybir as mybir

    nc = bacc.Bacc("TRN2", target_bir_lowering=False, debug=False, num_devices=8)
    xi = nc.dram_tensor("xi", [128, 16384], mybir.dt.bfloat16, kind="ExternalInput")
    wf = nc.dram_tensor("wf", [128, 12288], mybir.dt.bfloat16, kind="ExternalInput")
    wo = nc.dram_tensor("wo", [128, 4096], mybir.dt.bfloat16, kind="ExternalInput")
    out = nc.dram_tensor("out", [128, 16384], mybir.dt.float32, kind="ExternalOutput")
    with tile.TileContext(nc) as tc:
        build_kernel(tc, xi[:], wf[:], wo[:], out[:], reps=reps,
                     loop_iters=loop_iters, salt=salt)
    nc.compile()
    return nc


_NC_CACHE = {}


def _get_nc(reps=1, loop_iters=None, salt=""):
    key = (reps, loop_iters, salt)
    if key not in _NC_CACHE:
        _NC_CACHE[key] = build_bass(reps, loop_iters, salt)
    return _NC_CACHE[key]


def make_in_maps(x, W_qkv, W_o):
    tables = build_tables(W_qkv, W_o)
    x = np.asarray(x, dtype=np.float32)
    in_maps = []
    x_imgs = [pack_x(x[b]) for b in range(B)]
    for core in range(8):
        b = core // 2
        wf_img, wo_img = tables[core]
        in_maps.append({"xi": x_imgs[b], "wf": wf_img, "wo": wo_img})
    return in_maps


def combine_outputs(results):
    out = np.empty((B, S, M), dtype=np.float32)
    for b in range(B):
        acc = results[2 * b]["out"].astype(np.float32) + \
            results[2 * b + 1]["out"].astype(np.float32)
        # dev layout [128, sc*4096 + mo*512 + c] -> y[mo*128+p, sc*512+c]
        y = acc.reshape(128, 4, 8, 512).transpose(2, 0, 1, 3).reshape(1024, 2048)
        out[b] = y.T
    return out


def kernel(x, W_qkv, W_o):
    from concourse.bass_utils import run_bass_kernel_spmd
    nc = _get_nc()
    in_maps = make_in_maps(x, W_qkv, W_o)
    res = run_bass_kernel_spmd(nc, in_maps, core_ids=list(range(8)))
    return combine_outputs(res.results)


# revision 22
# speedup vs baseline: 1.0096x; 1.0096x over previous
"""HRR self-attention Trainium2 kernel.

Math: reference computes, per head (D=128):
    qkv = x @ W_qkv.T ; q,k,v heads
    kv  = irfft(rfft(k) * rfft(v))          # circular conv bind
    kv  = cumsum(kv, axis=seq)
    out = irfft(rfft(kv) * conj(rfft(q)))   # circular corr unbind
    y   = out @ W_o.T

The rfft/irfft along the head dim are linear maps, so they are folded into
W_qkv / W_o on the host: the device computes frequency-domain q,k,v directly
with one GEMM, does the complex bind / cumsum / unbind elementwise (the
cumsum commutes with the irfft), and applies the irfft+output projection as
a second GEMM. Zero extra FLOPs vs the plain projections.

Sharding: 8 cores = 4 batches x 2 head-groups (4 heads each). Each core
emits a partial output projection for its batch; host sums the two
head-group partials per batch.

Frequency packing per head-pair chunk (D=128 -> rfft bins 0..64): lanes
0..62 head A bins 1..63, lanes 63..125 head B bins 1..63, lanes 126/127
carry the real-only DC (re chunk) and Nyquist (im chunk) of heads A/B.
The special lanes need plain elementwise products, not complex ones; the
bind/unbind op ORDER makes that free: the two full-tile multiplies that
produce the correct special-lane values write the destination tiles first,
then the complex combines overwrite lanes [0:126] only. Same-engine program
order resolves the read-before-overwrite hazard. No fixup ops at all.

Inputs are host-packed into flat SBUF-image DRAM layouts so every DMA is a
contiguous per-partition run (max descriptor size, few instructions: each
DMACopy costs ~1.2us of SP issue + 625ns serialized HWDGE generation).
"""

import numpy as np
import ml_dtypes

B, S, M, H = 4, 2048, 1024, 8
D = M // H          # 128
SC = 512            # sequence chunk
NSC = S // SC       # 4
NMI = M // 128      # 8 contraction chunks
NCC = 12            # qkv freq channel chunks per core
NCO = 4             # U channel chunks per core

BF16 = ml_dtypes.bfloat16

# wf SBUF/DRAM image: col = h*6144 + mi*768 + cch*128 + c
WF_GROUPS = {0: [(0,), (1,), (2, 3), (4, 5), (6, 7)],
             1: [(0, 1, 2, 3), (4, 5, 6, 7)]}
# x image: sc0 region cols mi*512+c; rest region cols 4096 + g*6144 +
# idx*1536 + (sc-1)*512 + c
X0_GROUPS = [(0,), (1,), (2, 3), (4, 5), (6, 7)]


# ---------------------------------------------------------------------------
# Host-side weight fusion
# ---------------------------------------------------------------------------

def _head_rows(Wh, F):
    """Wh (D, M) spatial head weights -> (dc, nyq, re, im) freq rows."""
    FW = F @ Wh  # (65, M) complex
    return FW.real[0:1], FW.real[64:65], FW.real[1:64], FW.imag[1:64]


def build_tables(W_qkv, W_o):
    """Per-core (wf_img [128,12288] bf16, wo_img [128,4096] bf16)."""
    W_qkv = np.asarray(W_qkv, dtype=np.float64)
    W_o = np.asarray(W_o, dtype=np.float64)
    F = np.fft.rfft(np.eye(D), axis=-1).T  # (65, 128)
    Wq = W_qkv[0 * M:1 * M].reshape(H, D, M)
    Wk = W_qkv[1 * M:2 * M].reshape(H, D, M)
    Wv = W_qkv[2 * M:3 * M].reshape(H, D, M)

    # irfft basis columns, packed order [re 1..63 | dc]/[im 1..63 | nyq]
    n = np.arange(D)
    j = np.arange(1, 64)
    Bre = 2.0 * np.cos(2 * np.pi * np.outer(n, j) / D) / D   # (128, 63)
    Bim = -2.0 * np.sin(2 * np.pi * np.outer(n, j) / D) / D  # (128, 63)
    bdc = np.full((D, 1), 1.0 / D)
    bnyq = (np.cos(np.pi * n) / D)[:, None]

    tables = []
    for core in range(8):
        g = core % 2
        heads = [4 * g + i for i in range(4)]
        chunks = []
        out_rows = []
        for pair in range(2):
            hA, hB = heads[2 * pair], heads[2 * pair + 1]
            for Wx in (Wk, Wv, Wq):
                dcA, nyA, reA, imA = _head_rows(Wx[hA], F)
                dcB, nyB, reB, imB = _head_rows(Wx[hB], F)
                chunks.append(np.concatenate([reA, reB, dcA, dcB], axis=0))
                chunks.append(np.concatenate([imA, imB, nyA, nyB], axis=0))
            WoA = W_o[:, D * hA:D * (hA + 1)]  # (1024, 128)
            WoB = W_o[:, D * hB:D * (hB + 1)]
            out_rows.append(np.concatenate(
                [(WoA @ Bre).T, (WoB @ Bre).T, (WoA @ bdc).T, (WoB @ bdc).T],
                axis=0))
            out_rows.append(np.concatenate(
                [(WoA @ Bim).T, (WoB @ Bim).T, (WoA @ bnyq).T, (WoB @ bnyq).T],
                axis=0))
        WfT = np.concatenate(chunks, axis=0).T  # (1024, 1536)
        WoG = np.concatenate(out_rows, axis=0)  # (512, 1024)
        wf_img = np.ascontiguousarray(
            WfT.reshape(8, 128, 2, 768).transpose(1, 2, 0, 3).reshape(128, 12288))
        wo_img = np.ascontiguousarray(
            WoG.reshape(4, 128, 1024).transpose(1, 0, 2).reshape(128, 4096))
        tables.append((wf_img.astype(np.float32).astype(BF16),
                       wo_img.astype(np.float32).astype(BF16)))
    return tables


def pack_x(xb):
    """xb (S, M) f32 -> x image [128, 16384] bf16."""
    xT = np.ascontiguousarray(xb.T).astype(BF16)  # (1024, 2048)
    x0 = xT.reshape(8, 128, 4, 512)[:, :, 0]      # (8, 128, 512)
    x0 = x0.transpose(1, 0, 2).reshape(128, 4096)
    xr = xT[:, 512:].reshape(2, 4, 128, 1536)
    xr = xr.transpose(2, 0, 1, 3).reshape(128, 12288)
    return np.ascontiguousarray(np.concatenate([x0, xr], axis=1))


# ---------------------------------------------------------------------------
# Device kernel
# ---------------------------------------------------------------------------

def build_kernel(tc, xi, wf, wo, out, reps=1, loop_iters=None, salt=""):
    import concourse.mybir as mybir
    from contextlib import ExitStack

    nc = tc.nc
    bf16 = mybir.dt.bfloat16
    f32 = mybir.dt.float32
    MULT = mybir.AluOpType.mult
    ADD = mybir.AluOpType.add

    with ExitStack() as ctx:
        consts = ctx.enter_context(tc.tile_pool(name="consts", bufs=1))
        xpool = ctx.enter_context(tc.tile_pool(name="xpool", bufs=1))
        wpool = ctx.enter_context(tc.tile_pool(name="wpool", bufs=1))
        qkvp = ctx.enter_context(tc.tile_pool(name="qkvp", bufs=2))
        kvp = ctx.enter_context(tc.tile_pool(name="kvp", bufs=2))
        scanp = ctx.enter_context(tc.tile_pool(name="scanp", bufs=2))
        up = ctx.enter_context(tc.tile_pool(name="up", bufs=2))
        tmpp = ctx.enter_context(tc.tile_pool(name="tmpp", bufs=2))
        outp = ctx.enter_context(tc.tile_pool(name="outp", bufs=2))
        psq = ctx.enter_context(tc.tile_pool(name="psq", bufs=4, space="PSUM"))
        psop = ctx.enter_context(tc.tile_pool(name="psop", bufs=4, space="PSUM"))

        ones = consts.tile([128, SC], bf16, name=f"ones{salt}")
        nc.vector.memset(ones[:], 1.0)

        # --- input DMAs: few, contiguous, first-needed first ---
        wf_t = {0: [], 1: []}
        x0_t = []
        xr_t = []

        def wf_dma(h, gi):
            grp = WF_GROUPS[h][gi]
            w = len(grp) * 768
            t = wpool.tile([128, w], bf16, tag=f"wf{h}_{gi}", name=f"wf{h}_{gi}")
            c0 = h * 6144 + grp[0] * 768
            nc.sync.dma_start(out=t[:], in_=wf[:, c0:c0 + w])
            wf_t[h].append(t)

        def x0_dma(gi):
            grp = X0_GROUPS[gi]
            w = len(grp) * 512
            t = xpool.tile([128, w], bf16, tag=f"x0_{gi}", name=f"x0_{gi}")
            c0 = grp[0] * 512
            nc.sync.dma_start(out=t[:], in_=xi[:, c0:c0 + w])
            x0_t.append(t)

        for i in range(max(len(WF_GROUPS[0]), len(X0_GROUPS))):
            if i < len(WF_GROUPS[0]):
                wf_dma(0, i)
            if i < len(X0_GROUPS):
                x0_dma(i)
        for i in range(len(WF_GROUPS[1])):
            wf_dma(1, i)
        for gi in range(2):
            t = xpool.tile([128, 6144], bf16, tag=f"xr{gi}", name=f"xr{gi}")
            c0 = 4096 + gi * 6144
            nc.sync.dma_start(out=t[:], in_=xi[:, c0:c0 + 6144])
            xr_t.append(t)
        wo_t = wpool.tile([128, 4096], bf16, tag="wo", name="wo")
        nc.sync.dma_start(out=wo_t[:], in_=wo[:, :])

        # loop builds: fixed carry tiles let the last chunk's output GEMM
        # move to the TOP of the next For_i iteration (reads the previous
        # iteration's U), so the body never stalls on its own tail chain.
        ucarry = None
        if loop_iters is not None:
            ucarry = [consts.tile([128, SC], bf16, name=f"uc{i}{salt}")
                      for i in range(4)]
            for t in ucarry:
                nc.vector.memset(t[:], 1.0)

        def wf_ap(mi, h, cch):
            for gi, grp in enumerate(WF_GROUPS[h]):
                if mi in grp:
                    c = (mi - grp[0]) * 768 + cch * 128
                    return wf_t[h][gi][:, c:c + 128]
            raise AssertionError

        def x_ap(mi, sc):
            if sc == 0:
                for gi, grp in enumerate(X0_GROUPS):
                    if mi in grp:
                        c = (mi - grp[0]) * 512
                        return x0_t[gi][:, c:c + 512]
                raise AssertionError
            gi, idx = divmod(mi, 4)
            c = idx * 1536 + (sc - 1) * 512
            return xr_t[gi][:, c:c + 512]

        if loop_iters is not None:
            loop_cm = tc.For_i(
                0, loop_iters, 1,
                hint_engines=(mybir.EngineType.PE, mybir.EngineType.DVE,
                              mybir.EngineType.Activation, mybir.EngineType.Pool,
                              mybir.EngineType.SP))
            loop_cm.__enter__()

        CPLX = slice(0, 126)  # complex lanes; 126/127 are real DC/Nyq lanes

        def drain_qkv(rep, sc, cc, ps, chunks, tail):
            sb = qkvp.tile([128, SC], bf16, tag=f"qkv{cc}", name=f"qkv{rep}_{sc}_{cc}")
            # ACT drains keep the DVE free for the bind/scan/unbind chain;
            # on the tail chunk alternate ACT/DVE so the final chain starts
            # sooner (gpsimd has no PSUM port)
            if tail and cc % 2 == 1:
                nc.vector.tensor_copy(sb[:], ps[:])
            else:
                nc.scalar.copy(sb[:], ps[:])
            chunks.append(sb)

        def emit_qkv_vector(rep, sc, prev_scan, tail=False, uout=None):
            chunks = []
            if sc == 0 and loop_iters is None:
                # Startup: x/wf tiles are still streaming in. mi-outer over
                # six h=0 accumulators (4 psq + 2 borrowed psop banks) lets
                # the PE consume each (wf, x) mi-group as it lands instead
                # of stalling inside chunk 0's accumulation.
                ps6 = [psq.tile([128, SC], f32, tag="psq", name=f"psq{rep}_0_{c}")
                       for c in range(4)]
                ps6 += [psop.tile([128, SC], f32, tag="pso", name=f"psq{rep}_0_{c}")
                        for c in range(4, 6)]
                for mi in range(NMI):
                    for cc in range(6):
                        nc.tensor.matmul(
                            ps6[cc][:], wf_ap(mi, 0, cc), x_ap(mi, 0),
                            start=(mi == 0), stop=(mi == NMI - 1))
                for cc in range(6):
                    drain_qkv(rep, sc, cc, ps6[cc], chunks, tail)
                rest = range(6, NCC)
            else:
                rest = range(NCC)
            for cc in rest:
                ps = psq.tile([128, SC], f32, tag="psq", name=f"psq{rep}_{sc}_{cc}")
                h, cch = divmod(cc, 6)
                for mi in range(NMI):
                    nc.tensor.matmul(
                        ps[:], wf_ap(mi, h, cch), x_ap(mi, sc),
                        start=(mi == 0), stop=(mi == NMI - 1))
                drain_qkv(rep, sc, cc, ps, chunks, tail)

            U = []
            for pair in range(2):
                Kre, Kim, Vre, Vim, Qre, Qim = chunks[6 * pair:6 * pair + 6]

                KVre = kvp.tile([128, SC], bf16, tag=f"kvre{pair}", name=f"kvre{sc}_{pair}")
                KVim = kvp.tile([128, SC], bf16, tag=f"kvim{pair}", name=f"kvim{sc}_{pair}")
                t3 = tmpp.tile([128, SC], bf16, tag="t3", name=f"t3_{sc}_{pair}")
                t4 = tmpp.tile([128, SC], bf16, tag="t4", name=f"t4_{sc}_{pair}")
                # lanes 126/127 of KVre/KVim are final after the first two
                # muls (real DC/Nyq products); the complex combines then
                # overwrite lanes [0:126] only. Program order on the DVE
                # queue makes the read-then-overwrite of KVim safe.
                nc.vector.tensor_mul(KVre[:], Kre[:], Vre[:])
                nc.vector.tensor_mul(KVim[:], Kim[:], Vim[:])
                nc.vector.tensor_sub(KVre[CPLX], KVre[CPLX], KVim[CPLX])
                nc.vector.tensor_mul(t3[:], Kre[:], Vim[:])
                nc.vector.tensor_mul(t4[:], Kim[:], Vre[:])
                nc.vector.tensor_add(KVim[CPLX], t3[CPLX], t4[CPLX])

                # bf16 scan output: the scan state stays fp32 internally;
                # 16-bit operands put the four unbind muls in the DVE's 2x
                # perf mode (validated: rel err 5.8e-3 vs 5.3e-3 at f32)
                KVre_c = scanp.tile([128, SC], bf16, tag=f"scre{pair}", name=f"scre{sc}_{pair}")
                KVim_c = scanp.tile([128, SC], bf16, tag=f"scim{pair}", name=f"scim{sc}_{pair}")
                init_re = 0.0 if sc == 0 else prev_scan[(pair, 0)][:, SC - 1:SC]
                init_im = 0.0 if sc == 0 else prev_scan[(pair, 1)][:, SC - 1:SC]
                nc.vector.tensor_tensor_scan(
                    KVre_c[:], ones[:], KVre[:], init_re, MULT, ADD)
                nc.vector.tensor_tensor_scan(
                    KVim_c[:], ones[:], KVim[:], init_im, MULT, ADD)
                prev_scan[(pair, 0)] = KVre_c
                prev_scan[(pair, 1)] = KVim_c

                if uout is not None:
                    Ure, Uim = uout[2 * pair], uout[2 * pair + 1]
                else:
                    Ure = up.tile([128, SC], bf16, tag=f"ure{pair}", name=f"ure{sc}_{pair}")
                    Uim = up.tile([128, SC], bf16, tag=f"uim{pair}", name=f"uim{sc}_{pair}")
                u3 = tmpp.tile([128, SC], bf16, tag="u3", name=f"u3_{sc}_{pair}")
                u4 = tmpp.tile([128, SC], bf16, tag="u4", name=f"u4_{sc}_{pair}")
                # same trick: Uim first holds Qim*KVim (lanes 126/127 final),
                # Ure holds Qre*KVre; complex lanes are then overwritten.
                nc.vector.tensor_mul(Uim[:], KVim_c[:], Qim[:])
                nc.vector.tensor_mul(Ure[:], KVre_c[:], Qre[:])
                nc.vector.tensor_add(Ure[CPLX], Ure[CPLX], Uim[CPLX])
                nc.vector.tensor_mul(u3[:], KVim_c[:], Qre[:])
                nc.vector.tensor_mul(u4[:], KVre_c[:], Qim[:])
                nc.vector.tensor_sub(Uim[CPLX], u3[CPLX], u4[CPLX])
                U += [Ure, Uim]
            return U

        def emit_out(rep, sc, U, final=False):
            wide = outp.tile([128, 4096], f32, tag="wide", name=f"wide{rep}_{sc}")
            for half in range(2):
                po = [psop.tile([128, SC], f32, tag="pso",
                                name=f"pso{rep}_{sc}_{half}_{q}") for q in range(4)]
                # ci-outer: the ci=0 matmuls only need U[0], so the PE can
                # start the output GEMM as soon as the first U chunk lands.
                for ci in range(NCO):
                    for q in range(4):
                        mo = half * 4 + q
                        nc.tensor.matmul(
                            po[q][:], wo_t[:, ci * 1024 + mo * 128:ci * 1024 + (mo + 1) * 128],
                            U[ci][:], start=(ci == 0), stop=(ci == NCO - 1))
                if not final:
                    for q in range(4):
                        mo = half * 4 + q
                        nc.scalar.copy(wide[:, mo * 512:(mo + 1) * 512], po[q][:])
                    nc.sync.dma_start(
                        out=out[:, sc * 4096 + half * 2048:sc * 4096 + (half + 1) * 2048],
                        in_=wide[:, half * 2048:(half + 1) * 2048])
                else:
                    # tail drain: alternate copy engines + per-pair DMAs so
                    # the final writeback pipelines instead of serializing.
                    for q2 in range(2):
                        for q in (q2 * 2, q2 * 2 + 1):
                            mo = half * 4 + q
                            if q % 2 == 0:
                                nc.scalar.copy(wide[:, mo * 512:(mo + 1) * 512], po[q][:])
                            else:
                                nc.vector.tensor_copy(wide[:, mo * 512:(mo + 1) * 512], po[q][:])
                        mo0 = half * 4 + q2 * 2
                        nc.sync.dma_start(
                            out=out[:, sc * 4096 + mo0 * 512:sc * 4096 + (mo0 + 2) * 512],
                            in_=wide[:, mo0 * 512:(mo0 + 2) * 512])

        # Software pipelining: emit s-chunk sc+1's qkv matmuls BEFORE s-chunk
        # sc's output matmuls. The PE executes its queue in order, so this
        # keeps it streaming qkv work while the DVE bind/scan/unbind chain for
        # the previous chunk produces U. The pending-out carries ACROSS reps
        # so the PE starts the next rep's qkv instead of stalling on the last
        # chunk's chain at each iteration boundary.
        pend = None
        if ucarry is not None:
            # consumes the PREVIOUS For_i iteration's carry (memset-seeded
            # on the first pass); keeps the PE busy through the loop wrap.
            emit_out(reps - 1, NSC - 1, ucarry, final=True)
        for rep in range(reps):
            prev_scan = {}
            for sc in range(NSC):
                last = (rep == reps - 1 and sc == NSC - 1)
                U = emit_qkv_vector(rep, sc, prev_scan, tail=last,
                                    uout=(ucarry if (last and ucarry is not None)
                                          else None))
                if pend is not None:
                    emit_out(*pend)
                pend = (rep, sc, U)
        if ucarry is None:
            emit_out(*pend, final=True)
        if loop_iters is not None:
            loop_cm.__exit__(None, None, None)


def build_bass(reps=1, loop_iters=None, salt=""):
    import concourse.bacc as bacc
    import concourse.tile as tile
    import concourse.mybir as mybir

    nc = bacc.Bacc("TRN2", target_bir_lowering=False, debug=False, num_devices=8)
    xi = nc.dram_tensor("xi", [128, 16384], mybir.dt.bfloat16, kind="ExternalInput")
    wf = nc.dram_tensor("wf", [128, 12288], mybir.dt.bfloat16, kind="ExternalInput")
    wo = nc.dram_tensor("wo", [128, 4096], mybir.dt.bfloat16, kind="ExternalInput")
    out = nc.dram_tensor("out", [128, 16384], mybir.dt.float32, kind="ExternalOutput")
    with tile.TileContext(nc) as tc:
        build_kernel(tc, xi[:], wf[:], wo[:], out[:], reps=reps,
                     loop_iters=loop_iters, salt=salt)
    nc.compile()
    return nc


_NC_CACHE = {}


def _get_nc(reps=1, loop_iters=None, salt=""):
    key = (reps, loop_iters, salt)
    if key not in _NC_CACHE:
        _NC_CACHE[key] = build_bass(reps, loop_iters, salt)
    return _NC_CACHE[key]


def make_in_maps(x, W_qkv, W_o):
    tables = build_tables(W_qkv, W_o)
    x = np.asarray(x, dtype=np.float32)
    in_maps = []
    x_imgs = [pack_x(x[b]) for b in range(B)]
    for core in range(8):
        b = core // 2
        wf_img, wo_img = tables[core]
        in_maps.append({"xi": x_imgs[b], "wf": wf_img, "wo": wo_img})
    return in_maps


def combine_outputs(results):
    out = np.empty((B, S, M), dtype=np.float32)
    for b in range(B):
        acc = results[2 * b]["out"].astype(np.float32) + \
            results[2 * b + 1]["out"].astype(np.float32)
        # dev layout [128, sc*4096 + mo*512 + c] -> y[mo*128+p, sc*512+c]
        y = acc.reshape(128, 4, 8, 512).transpose(2, 0, 1, 3).reshape(1024, 2048)
        out[b] = y.T
    return out


def kernel(x, W_qkv, W_o):
    from concourse.bass_utils import run_bass_kernel_spmd
    nc = _get_nc()
    in_maps = make_in_maps(x, W_qkv, W_o)
    res = run_bass_kernel_spmd(nc, in_maps, core_ids=list(range(8)))
    return combine_outputs(res.results)


# revision 23
# speedup vs baseline: 1.7793x; 1.7624x over previous
"""HRR self-attention Trainium2 kernel.

Math: reference computes, per head (D=128):
    qkv = x @ W_qkv.T ; q,k,v heads
    kv  = irfft(rfft(k) * rfft(v))          # circular conv bind
    kv  = cumsum(kv, axis=seq)
    out = irfft(rfft(kv) * conj(rfft(q)))   # circular corr unbind
    y   = out @ W_o.T

The rfft/irfft along the head dim are linear maps, so they are folded into
W_qkv / W_o on the host: the device computes frequency-domain q,k,v directly
with one GEMM, does the complex bind / cumsum / unbind elementwise (the
cumsum commutes with the irfft), and applies the irfft+output projection as
a second GEMM. Zero extra FLOPs vs the plain projections.

Sharding: 8 cores = 4 batches x 2 head-groups (4 heads each). Each core
emits a partial output projection for its batch; host sums the two
head-group partials per batch.

Frequency packing per head-pair chunk (D=128 -> rfft bins 0..64): lanes
0..62 head A bins 1..63, lanes 63..125 head B bins 1..63, lanes 126/127
carry the real-only DC (re chunk) and Nyquist (im chunk) of heads A/B.
The special lanes need plain elementwise products, not complex ones; the
bind/unbind op ORDER makes that free: the two full-tile multiplies that
produce the correct special-lane values write the destination tiles first,
then the complex combines overwrite lanes [0:126] only. Same-engine program
order resolves the read-before-overwrite hazard. No fixup ops at all.

Inputs are host-packed into flat SBUF-image DRAM layouts so every DMA is a
contiguous per-partition run (max descriptor size, few instructions: each
DMACopy costs ~1.2us of SP issue + 625ns serialized HWDGE generation).
"""

import numpy as np
import ml_dtypes

B, S, M, H = 4, 2048, 1024, 8
D = M // H          # 128
SC = 512            # sequence chunk
NSC = S // SC       # 4
NMI = M // 128      # 8 contraction chunks
NCC = 12            # qkv freq channel chunks per core
NCO = 4             # U channel chunks per core

BF16 = ml_dtypes.bfloat16

# wf SBUF/DRAM image: col = h*6144 + mi*768 + cch*128 + c
WF_GROUPS = {0: [(0,), (1,), (2, 3), (4, 5), (6, 7)],
             1: [(0, 1, 2, 3), (4, 5, 6, 7)]}
# x image: sc0 region cols mi*512+c; rest region cols 4096 + g*6144 +
# idx*1536 + (sc-1)*512 + c
X0_GROUPS = [(0,), (1,), (2, 3), (4, 5), (6, 7)]


# ---------------------------------------------------------------------------
# Host-side weight fusion
# ---------------------------------------------------------------------------

def _head_rows(Wh, F):
    """Wh (D, M) spatial head weights -> (dc, nyq, re, im) freq rows."""
    FW = F @ Wh  # (65, M) complex
    return FW.real[0:1], FW.real[64:65], FW.real[1:64], FW.imag[1:64]


def build_tables(W_qkv, W_o):
    """Per-core (wf_img [128,12288] bf16, wo_img [128,4096] bf16)."""
    W_qkv = np.asarray(W_qkv, dtype=np.float64)
    W_o = np.asarray(W_o, dtype=np.float64)
    F = np.fft.rfft(np.eye(D), axis=-1).T  # (65, 128)
    Wq = W_qkv[0 * M:1 * M].reshape(H, D, M)
    Wk = W_qkv[1 * M:2 * M].reshape(H, D, M)
    Wv = W_qkv[2 * M:3 * M].reshape(H, D, M)

    # irfft basis columns, packed order [re 1..63 | dc]/[im 1..63 | nyq]
    n = np.arange(D)
    j = np.arange(1, 64)
    Bre = 2.0 * np.cos(2 * np.pi * np.outer(n, j) / D) / D   # (128, 63)
    Bim = -2.0 * np.sin(2 * np.pi * np.outer(n, j) / D) / D  # (128, 63)
    bdc = np.full((D, 1), 1.0 / D)
    bnyq = (np.cos(np.pi * n) / D)[:, None]

    tables = []
    for core in range(8):
        g = core % 2
        heads = [4 * g + i for i in range(4)]
        chunks = []
        out_rows = []
        for pair in range(2):
            hA, hB = heads[2 * pair], heads[2 * pair + 1]
            for Wx in (Wk, Wv, Wq):
                dcA, nyA, reA, imA = _head_rows(Wx[hA], F)
                dcB, nyB, reB, imB = _head_rows(Wx[hB], F)
                chunks.append(np.concatenate([reA, reB, dcA, dcB], axis=0))
                chunks.append(np.concatenate([imA, imB, nyA, nyB], axis=0))
            WoA = W_o[:, D * hA:D * (hA + 1)]  # (1024, 128)
            WoB = W_o[:, D * hB:D * (hB + 1)]
            out_rows.append(np.concatenate(
                [(WoA @ Bre).T, (WoB @ Bre).T, (WoA @ bdc).T, (WoB @ bdc).T],
                axis=0))
            out_rows.append(np.concatenate(
                [(WoA @ Bim).T, (WoB @ Bim).T, (WoA @ bnyq).T, (WoB @ bnyq).T],
                axis=0))
        WfT = np.concatenate(chunks, axis=0).T  # (1024, 1536)
        WoG = np.concatenate(out_rows, axis=0)  # (512, 1024)
        wf_img = np.ascontiguousarray(
            WfT.reshape(8, 128, 2, 768).transpose(1, 2, 0, 3).reshape(128, 12288))
        wo_img = np.ascontiguousarray(
            WoG.reshape(4, 128, 1024).transpose(1, 0, 2).reshape(128, 4096))
        tables.append((wf_img.astype(np.float32).astype(BF16),
                       wo_img.astype(np.float32).astype(BF16)))
    return tables


def pack_x(xb):
    """xb (S, M) f32 -> x image [128, 16384] bf16."""
    xT = np.ascontiguousarray(xb.T).astype(BF16)  # (1024, 2048)
    x0 = xT.reshape(8, 128, 4, 512)[:, :, 0]      # (8, 128, 512)
    x0 = x0.transpose(1, 0, 2).reshape(128, 4096)
    xr = xT[:, 512:].reshape(2, 4, 128, 1536)
    xr = xr.transpose(2, 0, 1, 3).reshape(128, 12288)
    return np.ascontiguousarray(np.concatenate([x0, xr], axis=1))


# ---------------------------------------------------------------------------
# Device kernel
# ---------------------------------------------------------------------------

def build_kernel(tc, xi, wf, wo, out, reps=1, loop_iters=None, salt=""):
    import concourse.mybir as mybir
    from contextlib import ExitStack

    nc = tc.nc
    bf16 = mybir.dt.bfloat16
    f32 = mybir.dt.float32
    MULT = mybir.AluOpType.mult
    ADD = mybir.AluOpType.add

    with ExitStack() as ctx:
        consts = ctx.enter_context(tc.tile_pool(name="consts", bufs=1))
        xpool = ctx.enter_context(tc.tile_pool(name="xpool", bufs=1))
        wpool = ctx.enter_context(tc.tile_pool(name="wpool", bufs=1))
        qkvp = ctx.enter_context(tc.tile_pool(name="qkvp", bufs=2))
        kvp = ctx.enter_context(tc.tile_pool(name="kvp", bufs=2))
        scanp = ctx.enter_context(tc.tile_pool(name="scanp", bufs=2))
        up = ctx.enter_context(tc.tile_pool(name="up", bufs=2))
        tmpp = ctx.enter_context(tc.tile_pool(name="tmpp", bufs=2))
        outp = ctx.enter_context(tc.tile_pool(name="outp", bufs=2))
        psq = ctx.enter_context(tc.tile_pool(name="psq", bufs=4, space="PSUM"))
        psop = ctx.enter_context(tc.tile_pool(name="psop", bufs=4, space="PSUM"))

        ones = consts.tile([128, SC], bf16, name=f"ones{salt}")
        nc.vector.memset(ones[:], 1.0)

        # --- input DMAs: few, contiguous, first-needed first ---
        wf_t = {0: [], 1: []}
        x0_t = []
        xr_t = []

        def wf_dma(h, gi):
            grp = WF_GROUPS[h][gi]
            w = len(grp) * 768
            t = wpool.tile([128, w], bf16, tag=f"wf{h}_{gi}", name=f"wf{h}_{gi}")
            c0 = h * 6144 + grp[0] * 768
            nc.sync.dma_start(out=t[:], in_=wf[:, c0:c0 + w])
            wf_t[h].append(t)

        def x0_dma(gi):
            grp = X0_GROUPS[gi]
            w = len(grp) * 512
            t = xpool.tile([128, w], bf16, tag=f"x0_{gi}", name=f"x0_{gi}")
            c0 = grp[0] * 512
            nc.sync.dma_start(out=t[:], in_=xi[:, c0:c0 + w])
            x0_t.append(t)

        for i in range(max(len(WF_GROUPS[0]), len(X0_GROUPS))):
            if i < len(WF_GROUPS[0]):
                wf_dma(0, i)
            if i < len(X0_GROUPS):
                x0_dma(i)
        for i in range(len(WF_GROUPS[1])):
            wf_dma(1, i)
        for gi in range(2):
            t = xpool.tile([128, 6144], bf16, tag=f"xr{gi}", name=f"xr{gi}")
            c0 = 4096 + gi * 6144
            nc.sync.dma_start(out=t[:], in_=xi[:, c0:c0 + 6144])
            xr_t.append(t)
        wo_t = wpool.tile([128, 4096], bf16, tag="wo", name="wo")
        nc.sync.dma_start(out=wo_t[:], in_=wo[:, :])

        # loop builds: fixed carry tiles let the last chunk's output GEMM
        # move to the TOP of the next For_i iteration (reads the previous
        # iteration's U), so the body never stalls on its own tail chain.
        ucarry = None
        if loop_iters is not None:
            ucarry = [consts.tile([128, SC], bf16, name=f"uc{i}{salt}")
                      for i in range(4)]
            for t in ucarry:
                nc.vector.memset(t[:], 1.0)

        def wf_ap(mi, h, cch):
            for gi, grp in enumerate(WF_GROUPS[h]):
                if mi in grp:
                    c = (mi - grp[0]) * 768 + cch * 128
                    return wf_t[h][gi][:, c:c + 128]
            raise AssertionError

        def x_ap(mi, sc):
            if sc == 0:
                for gi, grp in enumerate(X0_GROUPS):
                    if mi in grp:
                        c = (mi - grp[0]) * 512
                        return x0_t[gi][:, c:c + 512]
                raise AssertionError
            gi, idx = divmod(mi, 4)
            c = idx * 1536 + (sc - 1) * 512
            return xr_t[gi][:, c:c + 512]

        if loop_iters is not None:
            loop_cm = tc.For_i(
                0, loop_iters, 1,
                hint_engines=(mybir.EngineType.PE, mybir.EngineType.DVE,
                              mybir.EngineType.Activation, mybir.EngineType.Pool,
                              mybir.EngineType.SP))
            loop_cm.__enter__()

        CPLX = slice(0, 126)  # complex lanes; 126/127 are real DC/Nyq lanes

        def drain_qkv(rep, sc, cc, ps, chunks, tail):
            sb = qkvp.tile([128, SC], bf16, tag=f"qkv{cc}", name=f"qkv{rep}_{sc}_{cc}")
            # alternate ACT/DVE drains: halves the copy latency gating each
            # pair's bind (the DVE copies queue while the PE still streams
            # the chunk's remaining matmuls; gpsimd has no PSUM port)
            if cc % 2 == 1:
                nc.vector.tensor_copy(sb[:], ps[:])
            else:
                nc.scalar.copy(sb[:], ps[:])
            chunks.append(sb)

        def emit_qkv_vector(rep, sc, prev_scan, tail=False, uout=None):
            chunks = []
            if sc == 0 and loop_iters is None:
                # Startup: x/wf tiles are still streaming in. mi-outer over
                # six h=0 accumulators (4 psq + 2 borrowed psop banks) lets
                # the PE consume each (wf, x) mi-group as it lands instead
                # of stalling inside chunk 0's accumulation.
                ps6 = [psq.tile([128, SC], f32, tag="psq", name=f"psq{rep}_0_{c}")
                       for c in range(4)]
                ps6 += [psop.tile([128, SC], f32, tag="pso", name=f"psq{rep}_0_{c}")
                        for c in range(4, 6)]
                for mi in range(NMI):
                    for cc in range(6):
                        nc.tensor.matmul(
                            ps6[cc][:], wf_ap(mi, 0, cc), x_ap(mi, 0),
                            start=(mi == 0), stop=(mi == NMI - 1))
                for cc in range(6):
                    drain_qkv(rep, sc, cc, ps6[cc], chunks, tail)
                rest = range(6, NCC)
            else:
                rest = range(NCC)
            for cc in rest:
                ps = psq.tile([128, SC], f32, tag="psq", name=f"psq{rep}_{sc}_{cc}")
                h, cch = divmod(cc, 6)
                for mi in range(NMI):
                    nc.tensor.matmul(
                        ps[:], wf_ap(mi, h, cch), x_ap(mi, sc),
                        start=(mi == 0), stop=(mi == NMI - 1))
                drain_qkv(rep, sc, cc, ps, chunks, tail)

            U = []
            for pair in range(2):
                Kre, Kim, Vre, Vim, Qre, Qim = chunks[6 * pair:6 * pair + 6]

                KVre = kvp.tile([128, SC], bf16, tag=f"kvre{pair}", name=f"kvre{sc}_{pair}")
                KVim = kvp.tile([128, SC], bf16, tag=f"kvim{pair}", name=f"kvim{sc}_{pair}")
                t3 = tmpp.tile([128, SC], bf16, tag="t3", name=f"t3_{sc}_{pair}")
                t4 = tmpp.tile([128, SC], bf16, tag="t4", name=f"t4_{sc}_{pair}")
                # lanes 126/127 of KVre/KVim are final after the first two
                # muls (real DC/Nyq products); the complex combines then
                # overwrite lanes [0:126] only. Program order on the DVE
                # queue makes the read-then-overwrite of KVim safe.
                nc.vector.tensor_mul(KVre[:], Kre[:], Vre[:])
                nc.vector.tensor_mul(KVim[:], Kim[:], Vim[:])
                nc.vector.tensor_sub(KVre[CPLX], KVre[CPLX], KVim[CPLX])
                nc.vector.tensor_mul(t3[:], Kre[:], Vim[:])
                nc.vector.tensor_mul(t4[:], Kim[:], Vre[:])
                nc.vector.tensor_add(KVim[CPLX], t3[CPLX], t4[CPLX])

                # bf16 scan output: the scan state stays fp32 internally;
                # 16-bit operands put the four unbind muls in the DVE's 2x
                # perf mode (validated: rel err 5.8e-3 vs 5.3e-3 at f32)
                KVre_c = scanp.tile([128, SC], bf16, tag=f"scre{pair}", name=f"scre{sc}_{pair}")
                KVim_c = scanp.tile([128, SC], bf16, tag=f"scim{pair}", name=f"scim{sc}_{pair}")
                init_re = 0.0 if sc == 0 else prev_scan[(pair, 0)][:, SC - 1:SC]
                init_im = 0.0 if sc == 0 else prev_scan[(pair, 1)][:, SC - 1:SC]
                nc.vector.tensor_tensor_scan(
                    KVre_c[:], ones[:], KVre[:], init_re, MULT, ADD)
                nc.vector.tensor_tensor_scan(
                    KVim_c[:], ones[:], KVim[:], init_im, MULT, ADD)
                prev_scan[(pair, 0)] = KVre_c
                prev_scan[(pair, 1)] = KVim_c

                if uout is not None:
                    Ure, Uim = uout[2 * pair], uout[2 * pair + 1]
                else:
                    Ure = up.tile([128, SC], bf16, tag=f"ure{pair}", name=f"ure{sc}_{pair}")
                    Uim = up.tile([128, SC], bf16, tag=f"uim{pair}", name=f"uim{sc}_{pair}")
                u3 = tmpp.tile([128, SC], bf16, tag="u3", name=f"u3_{sc}_{pair}")
                u4 = tmpp.tile([128, SC], bf16, tag="u4", name=f"u4_{sc}_{pair}")
                # same trick: Uim first holds Qim*KVim (lanes 126/127 final),
                # Ure holds Qre*KVre; complex lanes are then overwritten.
                nc.vector.tensor_mul(Uim[:], KVim_c[:], Qim[:])
                nc.vector.tensor_mul(Ure[:], KVre_c[:], Qre[:])
                nc.vector.tensor_add(Ure[CPLX], Ure[CPLX], Uim[CPLX])
                nc.vector.tensor_mul(u3[:], KVim_c[:], Qre[:])
                nc.vector.tensor_mul(u4[:], KVre_c[:], Qim[:])
                nc.vector.tensor_sub(Uim[CPLX], u3[CPLX], u4[CPLX])
                U += [Ure, Uim]
            return U

        def emit_out(rep, sc, U, final=False):
            wide = outp.tile([128, 4096], f32, tag="wide", name=f"wide{rep}_{sc}")
            for half in range(2):
                po = [psop.tile([128, SC], f32, tag="pso",
                                name=f"pso{rep}_{sc}_{half}_{q}") for q in range(4)]
                # ci-outer: the ci=0 matmuls only need U[0], so the PE can
                # start the output GEMM as soon as the first U chunk lands.
                for ci in range(NCO):
                    for q in range(4):
                        mo = half * 4 + q
                        nc.tensor.matmul(
                            po[q][:], wo_t[:, ci * 1024 + mo * 128:ci * 1024 + (mo + 1) * 128],
                            U[ci][:], start=(ci == 0), stop=(ci == NCO - 1))
                if not final:
                    for q in range(4):
                        mo = half * 4 + q
                        nc.scalar.copy(wide[:, mo * 512:(mo + 1) * 512], po[q][:])
                    nc.sync.dma_start(
                        out=out[:, sc * 4096 + half * 2048:sc * 4096 + (half + 1) * 2048],
                        in_=wide[:, half * 2048:(half + 1) * 2048])
                else:
                    # tail drain: alternate copy engines + per-pair DMAs so
                    # the final writeback pipelines instead of serializing.
                    for q2 in range(2):
                        for q in (q2 * 2, q2 * 2 + 1):
                            mo = half * 4 + q
                            if q % 2 == 0:
                                nc.scalar.copy(wide[:, mo * 512:(mo + 1) * 512], po[q][:])
                            else:
                                nc.vector.tensor_copy(wide[:, mo * 512:(mo + 1) * 512], po[q][:])
                        mo0 = half * 4 + q2 * 2
                        nc.sync.dma_start(
                            out=out[:, sc * 4096 + mo0 * 512:sc * 4096 + (mo0 + 2) * 512],
                            in_=wide[:, mo0 * 512:(mo0 + 2) * 512])

        # Software pipelining: emit s-chunk sc+1's qkv matmuls BEFORE s-chunk
        # sc's output matmuls. The PE executes its queue in order, so this
        # keeps it streaming qkv work while the DVE bind/scan/unbind chain for
        # the previous chunk produces U. The pending-out carries ACROSS reps
        # so the PE starts the next rep's qkv instead of stalling on the last
        # chunk's chain at each iteration boundary.
        pend = None
        if ucarry is not None:
            # consumes the PREVIOUS For_i iteration's carry (memset-seeded
            # on the first pass); keeps the PE busy through the loop wrap.
            emit_out(reps - 1, NSC - 1, ucarry, final=True)
        for rep in range(reps):
            prev_scan = {}
            for sc in range(NSC):
                last = (rep == reps - 1 and sc == NSC - 1)
                U = emit_qkv_vector(rep, sc, prev_scan, tail=last,
                                    uout=(ucarry if (last and ucarry is not None)
                                          else None))
                if pend is not None:
                    emit_out(*pend)
                pend = (rep, sc, U)
        if ucarry is None:
            emit_out(*pend, final=True)
        if loop_iters is not None:
            loop_cm.__exit__(None, None, None)


def build_bass(reps=1, loop_iters=None, salt=""):
    import concourse.bacc as bacc
    import concourse.tile as tile
    import concourse.mybir as mybir

    nc = bacc.Bacc("TRN2", target_bir_lowering=False, debug=False, num_devices=8)
    xi = nc.dram_tensor("xi", [128, 16384], mybir.dt.bfloat16, kind="ExternalInput")
    wf = nc.dram_tensor("wf", [128, 12288], mybir.dt.bfloat16, kind="ExternalInput")
    wo = nc.dram_tensor("wo", [128, 4096], mybir.dt.bfloat16, kind="ExternalInput")
    out = nc.dram_tensor("out", [128, 16384], mybir.dt.float32, kind="ExternalOutput")
    with tile.TileContext(nc) as tc:
        build_kernel(tc, xi[:], wf[:], wo[:], out[:], reps=reps,
                     loop_iters=loop_iters, salt=salt)
    nc.compile()
    return nc


_NC_CACHE = {}


def _get_nc(reps=1, loop_iters=None, salt=""):
    key = (reps, loop_iters, salt)
    if key not in _NC_CACHE:
        _NC_CACHE[key] = build_bass(reps, loop_iters, salt)
    return _NC_CACHE[key]


def make_in_maps(x, W_qkv, W_o):
    tables = build_tables(W_qkv, W_o)
    x = np.asarray(x, dtype=np.float32)
    in_maps = []
    x_imgs = [pack_x(x[b]) for b in range(B)]
    for core in range(8):
        b = core // 2
        wf_img, wo_img = tables[core]
        in_maps.append({"xi": x_imgs[b], "wf": wf_img, "wo": wo_img})
    return in_maps


def combine_outputs(results):
    out = np.empty((B, S, M), dtype=np.float32)
    for b in range(B):
        acc = results[2 * b]["out"].astype(np.float32) + \
            results[2 * b + 1]["out"].astype(np.float32)
        # dev layout [128, sc*4096 + mo*512 + c] -> y[mo*128+p, sc*512+c]
        y = acc.reshape(128, 4, 8, 512).transpose(2, 0, 1, 3).reshape(1024, 2048)
        out[b] = y.T
    return out


def kernel(x, W_qkv, W_o):
    from concourse.bass_utils import run_bass_kernel_spmd
    nc = _get_nc()
    in_maps = make_in_maps(x, W_qkv, W_o)
    res = run_bass_kernel_spmd(nc, in_maps, core_ids=list(range(8)))
    return combine_outputs(res.results)
